# revision 41
# baseline (speedup 1.0000x reference)
"""Causal self-attention (B=4, T=2048, C=1024, H=16, D=64) on 8 trn2 NeuronCores.

Sharding: core c handles batch b = c//2 and head group g = c%2 (8 heads each).
Each core computes the qkv projection for its heads, causal flash attention,
and a partial output projection (its heads' rows of w_proj). The host sums the
two partials per batch.

Per-core kernel layout notes:
  - Host supplies x[b].T (so the contraction dim C lands on SBUF partitions),
    w_attn columns for its heads as [q|k|v] blocks, w_proj rows, and causal
    diagonal mask tiles. All bf16; PSUM accumulation fp32.
  - Q^T/K^T are built head-pair-packed: tile [128, T] = head A dims on
    partitions 0-63, head B on 64-127, so the two K=64 QK matmuls of a pair
    land in distinct PE row groups and overlap.
  - S^T tiles ([k, q] layout) get exp on ScalarE with the 1/sqrt(D) scale
    folded in; the softmax denominator falls out of an appended ones-column on
    V (M=65 AV matmul); the division is applied to y^T before the projection.
  - Work is ordered V -> per-pair (QT/KT -> attention) -> projection so the
    scalar/vector engines stream underneath the PE throughout; one PSUM pool
    (tags: av=2 banks, st=3x2 banks) covers every phase with no pool
    boundaries.
"""

import sys

sys.path.insert(0, "/opt/trn_rl_repo")

import numpy as np
import ml_dtypes

from concourse import bacc, mybir
import concourse.bass as bass
from concourse.tile import TileContext
from concourse.bass_utils import run_bass_kernel_spmd

B, T, C, H, D = 4, 2048, 1024, 16, 64
N_CORES = 8
HL = H // 2  # heads per core: 8
CL = HL * D  # local channels per core: 512
BF16 = mybir.dt.bfloat16
F32 = mybir.dt.float32
KT_TILES = C // 128  # 8 contraction tiles for the qkv projection
TT_TILES = T // 128  # 16 token tiles
QB = 4  # q blocks of 512


def build_program(unroll=1, diag_restrict=True, proj_interleave=True, tb_fuse=True):
    """unroll>1 repeats the whole compute body (for steady-state timing)."""
    nc = bacc.Bacc("TRN2", target_bir_lowering=False, debug=False, num_devices=N_CORES)
    xt = nc.dram_tensor("xt", [C, T], BF16, kind="ExternalInput")
    wa = nc.dram_tensor("wa", [C, 3 * CL], BF16, kind="ExternalInput")
    wp = nc.dram_tensor("wp", [CL, C], BF16, kind="ExternalInput")
    msk = nc.dram_tensor("msk", [128, 128], BF16, kind="ExternalInput")
    outp = nc.dram_tensor("outp", [T, C], F32, kind="ExternalOutput")

    with TileContext(nc) as tc:
        with tc.tile_pool(name="resid", bufs=1) as p_r, tc.tile_pool(
            name="ps", bufs=1, space="PSUM"
        ) as p_ps, tc.tile_pool(name="es", bufs=3) as p_es, tc.tile_pool(
            name="sc", bufs=4
        ) as p_sc, tc.tile_pool(name="ob", bufs=3) as p_ob:
            xt_sb = [p_r.tile([128, T], BF16, name=f"xt{i}", tag=f"xt{i}") for i in range(KT_TILES)]
            wa_sb = [p_r.tile([128, 3 * CL], BF16, name=f"wa{i}", tag=f"wa{i}") for i in range(KT_TILES)]
            wp_sb = [p_r.tile([128, C], BF16, name=f"wp{i}", tag=f"wp{i}") for i in range(4)]
            msk_sb = p_r.tile([128, 128], BF16, tag="msk")
            ones_sb = p_r.tile([128, 64], BF16, tag="ones")
            qt_sb = [p_r.tile([128, T], BF16, name=f"qt{p}", tag=f"qt{p}") for p in range(4)]
            kt_sb = [p_r.tile([128, T], BF16, name=f"kt{p}", tag=f"kt{p}") for p in range(4)]
            va_sb = [p_r.tile([128, HL * 65], BF16, name=f"va{i}", tag=f"va{i}") for i in range(TT_TILES)]
            yt_sb = [p_r.tile([128, T], BF16, name=f"yt{p}", tag=f"yt{p}") for p in range(4)]

            nc.sync.dma_start(out=msk_sb[:], in_=msk[:])
            nc.gpsimd.memset(ones_sb[:], 1.0)

            # HAM warmup: the PE clock-gate sits at 1.2 GHz until ~3.4us of
            # sustained activity. The input DMA ramp leaves the PE idle for
            # ~14us at kernel start, so burn it with dummy matmuls on a
            # memset tile (borrowing an "av" PSUM slot, which real work
            # doesn't need until the first attention block).
            warm = p_r.tile([128, 512], BF16, tag="warm")
            nc.gpsimd.memset(warm[:], 0.0)
            wps = p_ps.tile([128, 512], F32, tag="av", name="warmps", bufs=2)
            for _ in range(18):
                nc.tensor.matmul(
                    wps[:], lhsT=warm[:, 0:128], rhs=warm[:], start=True, stop=True
                )
            for i in range(KT_TILES):
                nc.sync.dma_start(out=xt_sb[i][:], in_=xt[i * 128 : (i + 1) * 128, :])
                nc.sync.dma_start(
                    out=wa_sb[i][:, 2 * CL : 3 * CL],
                    in_=wa[i * 128 : (i + 1) * 128, 2 * CL : 3 * CL],
                )
            for i in range(KT_TILES):
                nc.sync.dma_start(
                    out=wa_sb[i][:, 0 : 2 * CL], in_=wa[i * 128 : (i + 1) * 128, 0 : 2 * CL]
                )
            for i in range(4):
                nc.sync.dma_start(out=wp_sb[i][:], in_=wp[i * 128 : (i + 1) * 128, :])

            for _ in range(unroll):
                # ---- closure builders; each closure is a chunk of PE work
                # that can be interleaved into the attention kt loops so the
                # PE static order never runs dry while ScalarE paces exp.
                def v_closures(tt):
                    state = {}

                    def h1():
                        acc = p_ps.tile([128, 512], F32, tag="fill", name="accv", bufs=2)
                        state["acc"] = acc
                        for ki in range(4):
                            nc.tensor.matmul(
                                acc[:],
                                lhsT=xt_sb[ki][:, tt * 128 : (tt + 1) * 128],
                                rhs=wa_sb[ki][:, 2 * CL : 3 * CL],
                                start=(ki == 0),
                                stop=False,
                            )

                    def h2():
                        acc = state["acc"]
                        for ki in range(4, KT_TILES):
                            nc.tensor.matmul(
                                acc[:],
                                lhsT=xt_sb[ki][:, tt * 128 : (tt + 1) * 128],
                                rhs=wa_sb[ki][:, 2 * CL : 3 * CL],
                                start=False,
                                stop=(ki == KT_TILES - 1),
                            )
                        va_view = va_sb[tt].rearrange("p (h e) -> p h e", e=65)
                        nc.scalar.copy(
                            out=va_view[:, :, 0:64],
                            in_=acc[:].rearrange("p (h e) -> p h e", e=64),
                        )
                        nc.gpsimd.memset(va_view[:, :, 64:65], 1.0)

                    return [h1, h2]

                def accqk_closures(p, qb):
                    out = []
                    for dst, col0 in ((qt_sb, 0), (kt_sb, CL)):
                        state = {}

                        def h1(dst=dst, col0=col0, state=state):
                            acc = p_ps.tile([128, 512], F32, tag="fill", name="accqk", bufs=2)
                            state["acc"] = acc
                            for ki in range(4):
                                nc.tensor.matmul(
                                    acc[:],
                                    lhsT=wa_sb[ki][:, col0 + p * 128 : col0 + (p + 1) * 128],
                                    rhs=xt_sb[ki][:, qb * 512 : (qb + 1) * 512],
                                    start=(ki == 0),
                                    stop=False,
                                )

                        def h2(dst=dst, col0=col0, state=state):
                            acc = state["acc"]
                            for ki in range(4, KT_TILES):
                                nc.tensor.matmul(
                                    acc[:],
                                    lhsT=wa_sb[ki][:, col0 + p * 128 : col0 + (p + 1) * 128],
                                    rhs=xt_sb[ki][:, qb * 512 : (qb + 1) * 512],
                                    start=False,
                                    stop=(ki == KT_TILES - 1),
                                )
                            nc.scalar.copy(
                                out=dst[p][:, qb * 512 : (qb + 1) * 512], in_=acc[:]
                            )

                        out += [h1, h2]
                    return out

                def proj_closures(tt, split=False):
                    # split=True: first closures accumulate pairs 0-2 (no
                    # dependency on the last pair's division), the rest add
                    # pair 3 and store.
                    state = {}

                    def mk(nb, pps, last):
                        def h():
                            if nb == 0 and pps[0] == 0:
                                state["ob"] = p_ob.tile([128, C], F32, tag="ob", name="ob")
                            if pps[0] == 0:
                                state[nb] = p_ps.tile(
                                    [128, 512], F32, tag="fill", name="accp", bufs=2
                                )
                            acc = state[nb]
                            for pp in pps:
                                nc.tensor.matmul(
                                    acc[:],
                                    lhsT=yt_sb[pp][:, tt * 128 : (tt + 1) * 128],
                                    rhs=wp_sb[pp][:, nb * 512 : (nb + 1) * 512],
                                    start=(pp == 0),
                                    stop=(pp == 3),
                                )
                            if not last:
                                return
                            ob = state["ob"]
                            nc.vector.tensor_copy(
                                out=ob[:, nb * 512 : (nb + 1) * 512], in_=acc[:]
                            )
                            if nb == 1:
                                nc.sync.dma_start(
                                    out=outp[tt * 128 : (tt + 1) * 128, :], in_=ob[:]
                                )

                        return h

                    if not split:
                        return [mk(0, range(4), True), mk(1, range(4), True)]
                    return (
                        [mk(0, range(3), False), mk(1, range(3), False)],
                        [mk(0, [3], True), mk(1, [3], True)],
                    )

                blocks = [(p, qb) for p in range(4) for qb in range(QB)]

                # prologue: V tiles for q block 0 and pair 0's first QT/KT
                for tt in range(4):
                    for cl in v_closures(tt):
                        cl()
                for cl in accqk_closures(0, 0):
                    cl()

                last_final_tails = []
                for idx, (p, qb) in enumerate(blocks):
                    hA, hB = 2 * p, 2 * p + 1
                    filler = []
                    if idx + 1 < len(blocks):
                        filler += accqk_closures(*blocks[idx + 1])
                    if p == 0 and qb < QB - 1:
                        for tt in range(4 * (qb + 1), 4 * (qb + 1) + 4):
                            filler += v_closures(tt)
                    if p == 3 and qb > 0 and proj_interleave:
                        for tt in range(4 * (qb - 1), 4 * qb):
                            filler += proj_closures(tt)


                    filler_all = list(filler)
                    nkt = 4 * qb + 4
                    LAG = 3  # AV trails QK by this many k tiles
                    ya = p_ps.tile([128, 512], F32, tag="av", name="ya", bufs=2)
                    yb = p_ps.tile([128, 512], F32, tag="av", name="yb", bufs=2)
                    es_ring = {}

                    def emit_qk(kt, p=p, qb=qb):
                        # Diagonal tiles only need q columns >= kt*128; the
                        # causal triangle then only lives in the first 128 of
                        # the remaining columns. st/es keep head B at column
                        # 512 (fixed full-size tiles).
                        j = kt - 4 * qb
                        qoff = max(j, 0) * 128 if diag_restrict else 0
                        W = 512 - qoff
                        st = p_ps.tile([128, 1024], F32, tag="st", name="st", bufs=2)
                        nc.tensor.matmul(
                            st[:, 0:W],
                            lhsT=kt_sb[p][0:64, kt * 128 : (kt + 1) * 128],
                            rhs=qt_sb[p][0:64, qb * 512 + qoff : (qb + 1) * 512],
                            start=True,
                            stop=True,
                        )
                        nc.tensor.matmul(
                            st[:, 512 : 512 + W],
                            lhsT=kt_sb[p][64:128, kt * 128 : (kt + 1) * 128],
                            rhs=qt_sb[p][64:128, qb * 512 + qoff : (qb + 1) * 512],
                            start=True,
                            stop=True,
                        )
                        es = p_es.tile([128, 1024], BF16, tag="es", name="es", bufs=6)
                        if W == 512:
                            nc.scalar.activation(
                                out=es[:],
                                in_=st[:],
                                func=mybir.ActivationFunctionType.Exp,
                                scale=0.125,
                            )
                        else:
                            view_es = es.rearrange("p (h w) -> p h w", h=2)[:, :, 0:W]
                            view_st = st.rearrange("p (h w) -> p h w", h=2)[:, :, 0:W]
                            nc.scalar.activation(
                                out=view_es,
                                in_=view_st,
                                func=mybir.ActivationFunctionType.Exp,
                                scale=0.125,
                            )
                        if j >= 0:
                            nc.vector.tensor_tensor(
                                es[:, 0:128], es[:, 0:128], msk_sb[:], mybir.AluOpType.mult
                            )
                            nc.vector.tensor_tensor(
                                es[:, 512:640], es[:, 512:640], msk_sb[:], mybir.AluOpType.mult
                            )
                        es_ring[kt] = (es, qoff, W)

                    def emit_av(kt, p=p, qb=qb, nkt=nkt, ya=ya, yb=yb):
                        es, qoff, W = es_ring.pop(kt)
                        nc.tensor.matmul(
                            ya[0:65, qoff:512],
                            lhsT=va_sb[kt][:, hA * 65 : (hA + 1) * 65],
                            rhs=es[:, 0:W],
                            start=(kt == 0),
                            stop=(kt == nkt - 1),
                        )
                        nc.tensor.matmul(
                            yb[0:65, qoff:512],
                            lhsT=va_sb[kt][:, hB * 65 : (hB + 1) * 65],
                            rhs=es[:, 512 : 512 + W],
                            start=(kt == 0),
                            stop=(kt == nkt - 1),
                        )

                    total_iters = nkt + LAG
                    pops_done = 0
                    for kt in range(total_iters):
                        if kt < nkt:
                            emit_qk(kt)
                        if kt >= LAG:
                            emit_av(kt - LAG)
                        want = (kt + 1) * len(filler_all) // total_iters
                        while pops_done < want and filler:
                            filler.pop(0)()
                            pops_done += 1
                    while filler:
                        filler.pop(0)()

                    div_order = ((1, yb), (0, ya)) if idx == len(blocks) - 1 else ((0, ya), (1, yb))
                    for s, yy in div_order:
                        rec = p_sc.tile([128, 512], BF16, tag="rec", name="rec")
                        with nc.allow_low_precision(reason="softmax denom in bf16"):
                            nc.vector.reciprocal(out=rec[64:65, :], in_=yy[64:65, :])
                        rep = p_ps.tile([128, 512], F32, tag="fill", name="rep", bufs=2)
                        nc.tensor.matmul(
                            rep[0:64, :],
                            lhsT=ones_sb[64:65, 0:64],
                            rhs=rec[64:65, :],
                            start=True,
                            stop=True,
                        )
                        reps = p_sc.tile([128, 512], BF16, tag="reps", name="reps")
                        nc.vector.tensor_copy(out=reps[0:64, :], in_=rep[0:64, :])
                        if s == 0:
                            nc.vector.tensor_tensor(
                                yt_sb[p][0:64, qb * 512 : (qb + 1) * 512],
                                yy[0:64, :],
                                reps[0:64, :],
                                mybir.AluOpType.mult,
                            )
                        else:
                            tmp = p_sc.tile([128, 512], BF16, tag="ytmp", name="ytmp")
                            nc.vector.tensor_tensor(
                                tmp[0:64, :],
                                yy[0:64, :],
                                reps[0:64, :],
                                mybir.AluOpType.mult,
                            )
                            nc.sync.dma_start(
                                out=yt_sb[p][64:128, qb * 512 : (qb + 1) * 512],
                                in_=tmp[0:64, :],
                            )

                # final projection group (last q block of the last pair)
                for tt in range(4 * (QB - 1), 4 * QB):
                    for cl in proj_closures(tt):
                        cl()

    nc.compile()
    return nc


_NC_CACHE = None


def _get_program():
    global _NC_CACHE
    if _NC_CACHE is None:
        _NC_CACHE = build_program()
    return _NC_CACHE


def _make_masks():
    ki = np.arange(128)[:, None]
    qi = np.arange(128)[None, :]
    return (ki <= qi).astype(np.float32).astype(ml_dtypes.bfloat16)


def make_in_maps(x, w_attn, w_proj):
    msk = _make_masks()
    xts = [x[b].T.astype(ml_dtypes.bfloat16, order="C") for b in range(B)]
    was, wps = [], []
    for g in range(2):
        was.append(
            np.concatenate(
                [
                    w_attn[:, 512 * g : 512 * g + 512],
                    w_attn[:, C + 512 * g : C + 512 * g + 512],
                    w_attn[:, 2 * C + 512 * g : 2 * C + 512 * g + 512],
                ],
                axis=1,
            ).astype(ml_dtypes.bfloat16, order="C")
        )
        wps.append(
            w_proj[512 * g : 512 * g + 512, :].astype(ml_dtypes.bfloat16, order="C")
        )
    return [
        {"xt": xts[c // 2], "wa": was[c % 2], "wp": wps[c % 2], "msk": msk}
        for c in range(N_CORES)
    ]


def gather_output(results):
    out = np.empty((B, T, C), np.float32)
    for b in range(B):
        out[b] = results[2 * b]["outp"] + results[2 * b + 1]["outp"]
    return out


_RUNNER = None


def _make_cached_runner(nc):
    """jit the SPMD executable once so repeat kernel() calls skip the
    per-call retrace/recompile that run_bass_kernel_spmd pays."""
    import jax
    from jax.sharding import Mesh, PartitionSpec
    from jax.experimental.shard_map import shard_map
    from concourse import bass2jax

    bass2jax.install_neuronx_cc_hook()
    partition_name = nc.partition_id_tensor.name if nc.partition_id_tensor else None
    in_names, out_names, out_avals, zero_outs = [], [], [], []
    for alloc in nc.m.functions[0].allocations:
        if not isinstance(alloc, mybir.MemoryLocationSet):
            continue
        name = alloc.memorylocations[0].name
        if alloc.kind == "ExternalInput":
            if name != partition_name:
                in_names.append(name)
        elif alloc.kind == "ExternalOutput":
            shape = tuple(alloc.tensor_shape)
            dtype = mybir.dt.np(alloc.dtype)
            out_names.append(name)
            out_avals.append(jax.core.ShapedArray(shape, dtype))
            zero_outs.append(np.zeros(shape, dtype))
    n_params = len(in_names)
    n_outs = len(out_avals)
    all_in_names = in_names + out_names
    if partition_name is not None:
        all_in_names.append(partition_name)

    def _body(*args):
        operands = list(args)
        if partition_name is not None:
            operands.append(bass2jax.partition_id_tensor())
        return tuple(
            bass2jax._bass_exec_p.bind(
                *operands,
                out_avals=tuple(out_avals),
                in_names=tuple(all_in_names),
                out_names=tuple(out_names),
                lowering_input_output_aliases=(),
                sim_require_finite=True,
                sim_require_nnan=True,
                nc=nc,
            )
        )

    devices = jax.devices()[:N_CORES]
    mesh = Mesh(np.asarray(devices), ("core",))
    spec = PartitionSpec("core")
    sharded = jax.jit(
        shard_map(
            _body,
            mesh=mesh,
            in_specs=(spec,) * (n_params + n_outs),
            out_specs=(spec,) * n_outs,
            check_rep=False,
        ),
        donate_argnums=tuple(range(n_params, n_params + n_outs)),
        keep_unused=True,
    )

    def run(in_maps):
        concat_in = [
            np.concatenate([np.asarray(in_maps[c][nm]) for c in range(N_CORES)], 0)
            for nm in in_names
        ]
        zeros = [
            np.zeros((N_CORES * z.shape[0], *z.shape[1:]), z.dtype) for z in zero_outs
        ]
        outs = sharded(*concat_in, *zeros)
        return [
            {
                name: np.asarray(outs[i]).reshape(N_CORES, *out_avals[i].shape)[c]
                for i, name in enumerate(out_names)
            }
            for c in range(N_CORES)
        ]

    return run


def kernel(x, w_attn, w_proj):
    global _RUNNER
    x = np.asarray(x, dtype=np.float32)
    w_attn = np.asarray(w_attn, dtype=np.float32)
    w_proj = np.asarray(w_proj, dtype=np.float32)
    nc = _get_program()
    in_maps = make_in_maps(x, w_attn, w_proj)
    if _RUNNER is None:
        try:
            _RUNNER = _make_cached_runner(nc)
        except Exception:
            _RUNNER = None
        if _RUNNER is None:
            res = run_bass_kernel_spmd(nc, in_maps, core_ids=list(range(N_CORES)))
            return gather_output(res.results)
    try:
        return gather_output(_RUNNER(in_maps))
    except Exception:
        res = run_bass_kernel_spmd(nc, in_maps, core_ids=list(range(N_CORES)))
        return gather_output(res.results)


# revision 42
# speedup vs baseline: 1.0027x; 1.0027x over previous
"""Causal self-attention (B=4, T=2048, C=1024, H=16, D=64) on 8 trn2 NeuronCores.

Sharding: core c handles batch b = c//2 and head group g = c%2 (8 heads each).
Each core computes the qkv projection for its heads, causal flash attention,
and a partial output projection (its heads' rows of w_proj). The host sums the
two partials per batch.

Per-core kernel layout notes:
  - Host supplies x[b].T (so the contraction dim C lands on SBUF partitions),
    w_attn columns for its heads as [q|k|v] blocks, w_proj rows, and causal
    diagonal mask tiles. All bf16; PSUM accumulation fp32.
  - Q^T/K^T are built head-pair-packed: tile [128, T] = head A dims on
    partitions 0-63, head B on 64-127, so the two K=64 QK matmuls of a pair
    land in distinct PE row groups and overlap.
  - S^T tiles ([k, q] layout) get exp on ScalarE with the 1/sqrt(D) scale
    folded in; the softmax denominator falls out of an appended ones-column on
    V (M=65 AV matmul); the division is applied to y^T before the projection.
  - Work is ordered V -> per-pair (QT/KT -> attention) -> projection so the
    scalar/vector engines stream underneath the PE throughout; one PSUM pool
    (tags: av=2 banks, st=3x2 banks) covers every phase with no pool
    boundaries.
"""

import sys

sys.path.insert(0, "/opt/trn_rl_repo")

import numpy as np
import ml_dtypes

from concourse import bacc, mybir
import concourse.bass as bass
from concourse.tile import TileContext
from concourse.bass_utils import run_bass_kernel_spmd

B, T, C, H, D = 4, 2048, 1024, 16, 64
N_CORES = 8
HL = H // 2  # heads per core: 8
CL = HL * D  # local channels per core: 512
BF16 = mybir.dt.bfloat16
F32 = mybir.dt.float32
KT_TILES = C // 128  # 8 contraction tiles for the qkv projection
TT_TILES = T // 128  # 16 token tiles
QB = 4  # q blocks of 512


def build_program(unroll=1, diag_restrict=True, proj_interleave=True, tb_fuse=True):
    """unroll>1 repeats the whole compute body (for steady-state timing)."""
    nc = bacc.Bacc("TRN2", target_bir_lowering=False, debug=False, num_devices=N_CORES)
    xt = nc.dram_tensor("xt", [C, T], BF16, kind="ExternalInput")
    wa = nc.dram_tensor("wa", [C, 3 * CL], BF16, kind="ExternalInput")
    wp = nc.dram_tensor("wp", [CL, C], BF16, kind="ExternalInput")
    msk = nc.dram_tensor("msk", [128, 128], BF16, kind="ExternalInput")
    outp = nc.dram_tensor("outp", [T, C], F32, kind="ExternalOutput")

    with TileContext(nc) as tc:
        with tc.tile_pool(name="resid", bufs=1) as p_r, tc.tile_pool(
            name="ps", bufs=1, space="PSUM"
        ) as p_ps, tc.tile_pool(name="es", bufs=3) as p_es, tc.tile_pool(
            name="sc", bufs=4
        ) as p_sc, tc.tile_pool(name="ob", bufs=3) as p_ob:
            xt_sb = [p_r.tile([128, T], BF16, name=f"xt{i}", tag=f"xt{i}") for i in range(KT_TILES)]
            wa_sb = [p_r.tile([128, 3 * CL], BF16, name=f"wa{i}", tag=f"wa{i}") for i in range(KT_TILES)]
            wp_sb = [p_r.tile([128, C], BF16, name=f"wp{i}", tag=f"wp{i}") for i in range(4)]
            msk_sb = p_r.tile([128, 128], BF16, tag="msk")
            ones_sb = p_r.tile([128, 64], BF16, tag="ones")
            qt_sb = [p_r.tile([128, T], BF16, name=f"qt{p}", tag=f"qt{p}") for p in range(4)]
            kt_sb = [p_r.tile([128, T], BF16, name=f"kt{p}", tag=f"kt{p}") for p in range(4)]
            va_sb = [p_r.tile([128, HL * 65], BF16, name=f"va{i}", tag=f"va{i}") for i in range(TT_TILES)]
            yt_sb = [p_r.tile([128, T], BF16, name=f"yt{p}", tag=f"yt{p}") for p in range(4)]

            nc.sync.dma_start(out=msk_sb[:], in_=msk[:])
            nc.gpsimd.memset(ones_sb[:], 1.0)

            # HAM warmup: the PE clock-gate sits at 1.2 GHz until ~3.4us of
            # sustained activity. The input DMA ramp leaves the PE idle for
            # ~14us at kernel start, so burn it with dummy matmuls on a
            # memset tile (borrowing an "av" PSUM slot, which real work
            # doesn't need until the first attention block).
            warm = p_r.tile([128, 512], BF16, tag="warm")
            nc.gpsimd.memset(warm[:], 0.0)
            wps = p_ps.tile([128, 512], F32, tag="av", name="warmps", bufs=2)
            for _ in range(18):
                nc.tensor.matmul(
                    wps[:], lhsT=warm[:, 0:128], rhs=warm[:], start=True, stop=True
                )
            for i in range(KT_TILES):
                nc.sync.dma_start(out=xt_sb[i][:], in_=xt[i * 128 : (i + 1) * 128, :])
                nc.sync.dma_start(
                    out=wa_sb[i][:, 2 * CL : 3 * CL],
                    in_=wa[i * 128 : (i + 1) * 128, 2 * CL : 3 * CL],
                )
            for i in range(KT_TILES):
                nc.sync.dma_start(
                    out=wa_sb[i][:, 0 : 2 * CL], in_=wa[i * 128 : (i + 1) * 128, 0 : 2 * CL]
                )
            for i in range(4):
                nc.sync.dma_start(out=wp_sb[i][:], in_=wp[i * 128 : (i + 1) * 128, :])

            for _ in range(unroll):
                # ---- closure builders; each closure is a chunk of PE work
                # that can be interleaved into the attention kt loops so the
                # PE static order never runs dry while ScalarE paces exp.
                def v_closures(tt):
                    state = {}

                    def h1():
                        acc = p_ps.tile([128, 512], F32, tag="fill", name="accv", bufs=2)
                        state["acc"] = acc
                        for ki in range(4):
                            nc.tensor.matmul(
                                acc[:],
                                lhsT=xt_sb[ki][:, tt * 128 : (tt + 1) * 128],
                                rhs=wa_sb[ki][:, 2 * CL : 3 * CL],
                                start=(ki == 0),
                                stop=False,
                            )

                    def h2():
                        acc = state["acc"]
                        for ki in range(4, KT_TILES):
                            nc.tensor.matmul(
                                acc[:],
                                lhsT=xt_sb[ki][:, tt * 128 : (tt + 1) * 128],
                                rhs=wa_sb[ki][:, 2 * CL : 3 * CL],
                                start=False,
                                stop=(ki == KT_TILES - 1),
                            )
                        va_view = va_sb[tt].rearrange("p (h e) -> p h e", e=65)
                        nc.scalar.copy(
                            out=va_view[:, :, 0:64],
                            in_=acc[:].rearrange("p (h e) -> p h e", e=64),
                        )
                        nc.gpsimd.memset(va_view[:, :, 64:65], 1.0)

                    return [h1, h2]

                def accqk_closures(p, qb):
                    out = []
                    for dst, col0 in ((qt_sb, 0), (kt_sb, CL)):
                        state = {}

                        def h1(dst=dst, col0=col0, state=state):
                            acc = p_ps.tile([128, 512], F32, tag="fill", name="accqk", bufs=2)
                            state["acc"] = acc
                            for ki in range(4):
                                nc.tensor.matmul(
                                    acc[:],
                                    lhsT=wa_sb[ki][:, col0 + p * 128 : col0 + (p + 1) * 128],
                                    rhs=xt_sb[ki][:, qb * 512 : (qb + 1) * 512],
                                    start=(ki == 0),
                                    stop=False,
                                )

                        def h2(dst=dst, col0=col0, state=state):
                            acc = state["acc"]
                            for ki in range(4, KT_TILES):
                                nc.tensor.matmul(
                                    acc[:],
                                    lhsT=wa_sb[ki][:, col0 + p * 128 : col0 + (p + 1) * 128],
                                    rhs=xt_sb[ki][:, qb * 512 : (qb + 1) * 512],
                                    start=False,
                                    stop=(ki == KT_TILES - 1),
                                )
                            nc.scalar.copy(
                                out=dst[p][:, qb * 512 : (qb + 1) * 512], in_=acc[:]
                            )

                        out += [h1, h2]
                    return out

                def proj_closures(tt, split=False):
                    # split=True: first closures accumulate pairs 0-2 (no
                    # dependency on the last pair's division), the rest add
                    # pair 3 and store.
                    state = {}

                    def mk(nb, pps, last):
                        def h():
                            if nb == 0 and pps[0] == 0:
                                state["ob"] = p_ob.tile([128, C], F32, tag="ob", name="ob")
                            if pps[0] == 0:
                                state[nb] = p_ps.tile(
                                    [128, 512], F32, tag="fill", name="accp", bufs=2
                                )
                            acc = state[nb]
                            for pp in pps:
                                nc.tensor.matmul(
                                    acc[:],
                                    lhsT=yt_sb[pp][:, tt * 128 : (tt + 1) * 128],
                                    rhs=wp_sb[pp][:, nb * 512 : (nb + 1) * 512],
                                    start=(pp == 0),
                                    stop=(pp == 3),
                                )
                            if not last:
                                return
                            ob = state["ob"]
                            nc.vector.tensor_copy(
                                out=ob[:, nb * 512 : (nb + 1) * 512], in_=acc[:]
                            )
                            nc.sync.dma_start(
                                out=outp[
                                    tt * 128 : (tt + 1) * 128, nb * 512 : (nb + 1) * 512
                                ],
                                in_=ob[:, nb * 512 : (nb + 1) * 512],
                            )

                        return h

                    if not split:
                        return [mk(0, range(4), True), mk(1, range(4), True)]
                    return (
                        [mk(0, range(3), False), mk(1, range(3), False)],
                        [mk(0, [3], True), mk(1, [3], True)],
                    )

                blocks = [(p, qb) for p in range(4) for qb in range(QB)]

                # prologue: V tiles for q block 0 and pair 0's first QT/KT
                for tt in range(4):
                    for cl in v_closures(tt):
                        cl()
                for cl in accqk_closures(0, 0):
                    cl()

                last_final_tails = []
                for idx, (p, qb) in enumerate(blocks):
                    hA, hB = 2 * p, 2 * p + 1
                    filler = []
                    if idx + 1 < len(blocks):
                        filler += accqk_closures(*blocks[idx + 1])
                    if p == 0 and qb < QB - 1:
                        for tt in range(4 * (qb + 1), 4 * (qb + 1) + 4):
                            filler += v_closures(tt)
                    if p == 3 and qb > 0 and proj_interleave:
                        for tt in range(4 * (qb - 1), 4 * qb):
                            filler += proj_closures(tt)


                    filler_all = list(filler)
                    nkt = 4 * qb + 4
                    LAG = 3  # AV trails QK by this many k tiles
                    ya = p_ps.tile([128, 512], F32, tag="av", name="ya", bufs=2)
                    yb = p_ps.tile([128, 512], F32, tag="av", name="yb", bufs=2)
                    es_ring = {}

                    def emit_qk(kt, p=p, qb=qb):
                        # Diagonal tiles only need q columns >= kt*128; the
                        # causal triangle then only lives in the first 128 of
                        # the remaining columns. st/es keep head B at column
                        # 512 (fixed full-size tiles).
                        j = kt - 4 * qb
                        qoff = max(j, 0) * 128 if diag_restrict else 0
                        W = 512 - qoff
                        st = p_ps.tile([128, 1024], F32, tag="st", name="st", bufs=2)
                        nc.tensor.matmul(
                            st[:, 0:W],
                            lhsT=kt_sb[p][0:64, kt * 128 : (kt + 1) * 128],
                            rhs=qt_sb[p][0:64, qb * 512 + qoff : (qb + 1) * 512],
                            start=True,
                            stop=True,
                        )
                        nc.tensor.matmul(
                            st[:, 512 : 512 + W],
                            lhsT=kt_sb[p][64:128, kt * 128 : (kt + 1) * 128],
                            rhs=qt_sb[p][64:128, qb * 512 + qoff : (qb + 1) * 512],
                            start=True,
                            stop=True,
                        )
                        es = p_es.tile([128, 1024], BF16, tag="es", name="es", bufs=6)
                        if W == 512:
                            nc.scalar.activation(
                                out=es[:],
                                in_=st[:],
                                func=mybir.ActivationFunctionType.Exp,
                                scale=0.125,
                            )
                        else:
                            view_es = es.rearrange("p (h w) -> p h w", h=2)[:, :, 0:W]
                            view_st = st.rearrange("p (h w) -> p h w", h=2)[:, :, 0:W]
                            nc.scalar.activation(
                                out=view_es,
                                in_=view_st,
                                func=mybir.ActivationFunctionType.Exp,
                                scale=0.125,
                            )
                        if j >= 0:
                            nc.vector.tensor_tensor(
                                es[:, 0:128], es[:, 0:128], msk_sb[:], mybir.AluOpType.mult
                            )
                            nc.vector.tensor_tensor(
                                es[:, 512:640], es[:, 512:640], msk_sb[:], mybir.AluOpType.mult
                            )
                        es_ring[kt] = (es, qoff, W)

                    def emit_av(kt, p=p, qb=qb, nkt=nkt, ya=ya, yb=yb):
                        es, qoff, W = es_ring.pop(kt)
                        nc.tensor.matmul(
                            ya[0:65, qoff:512],
                            lhsT=va_sb[kt][:, hA * 65 : (hA + 1) * 65],
                            rhs=es[:, 0:W],
                            start=(kt == 0),
                            stop=(kt == nkt - 1),
                        )
                        nc.tensor.matmul(
                            yb[0:65, qoff:512],
                            lhsT=va_sb[kt][:, hB * 65 : (hB + 1) * 65],
                            rhs=es[:, 512 : 512 + W],
                            start=(kt == 0),
                            stop=(kt == nkt - 1),
                        )

                    total_iters = nkt + LAG
                    pops_done = 0
                    for kt in range(total_iters):
                        if kt < nkt:
                            emit_qk(kt)
                        if kt >= LAG:
                            emit_av(kt - LAG)
                        want = (kt + 1) * len(filler_all) // total_iters
                        while pops_done < want and filler:
                            filler.pop(0)()
                            pops_done += 1
                    while filler:
                        filler.pop(0)()

                    div_order = ((1, yb), (0, ya)) if idx == len(blocks) - 1 else ((0, ya), (1, yb))
                    for s, yy in div_order:
                        rec = p_sc.tile([128, 512], BF16, tag="rec", name="rec")
                        with nc.allow_low_precision(reason="softmax denom in bf16"):
                            nc.vector.reciprocal(out=rec[64:65, :], in_=yy[64:65, :])
                        rep = p_ps.tile([128, 512], F32, tag="fill", name="rep", bufs=2)
                        nc.tensor.matmul(
                            rep[0:64, :],
                            lhsT=ones_sb[64:65, 0:64],
                            rhs=rec[64:65, :],
                            start=True,
                            stop=True,
                        )
                        reps = p_sc.tile([128, 512], BF16, tag="reps", name="reps")
                        nc.vector.tensor_copy(out=reps[0:64, :], in_=rep[0:64, :])
                        if s == 0:
                            nc.vector.tensor_tensor(
                                yt_sb[p][0:64, qb * 512 : (qb + 1) * 512],
                                yy[0:64, :],
                                reps[0:64, :],
                                mybir.AluOpType.mult,
                            )
                        else:
                            tmp = p_sc.tile([128, 512], BF16, tag="ytmp", name="ytmp")
                            nc.vector.tensor_tensor(
                                tmp[0:64, :],
                                yy[0:64, :],
                                reps[0:64, :],
                                mybir.AluOpType.mult,
                            )
                            nc.sync.dma_start(
                                out=yt_sb[p][64:128, qb * 512 : (qb + 1) * 512],
                                in_=tmp[0:64, :],
                            )

                # final projection group (last q block of the last pair)
                for tt in range(4 * (QB - 1), 4 * QB):
                    for cl in proj_closures(tt):
                        cl()

    nc.compile()
    return nc


_NC_CACHE = None


def _get_program():
    global _NC_CACHE
    if _NC_CACHE is None:
        _NC_CACHE = build_program()
    return _NC_CACHE


def _make_masks():
    ki = np.arange(128)[:, None]
    qi = np.arange(128)[None, :]
    return (ki <= qi).astype(np.float32).astype(ml_dtypes.bfloat16)


def make_in_maps(x, w_attn, w_proj):
    msk = _make_masks()
    xts = [x[b].T.astype(ml_dtypes.bfloat16, order="C") for b in range(B)]
    was, wps = [], []
    for g in range(2):
        was.append(
            np.concatenate(
                [
                    w_attn[:, 512 * g : 512 * g + 512],
                    w_attn[:, C + 512 * g : C + 512 * g + 512],
                    w_attn[:, 2 * C + 512 * g : 2 * C + 512 * g + 512],
                ],
                axis=1,
            ).astype(ml_dtypes.bfloat16, order="C")
        )
        wps.append(
            w_proj[512 * g : 512 * g + 512, :].astype(ml_dtypes.bfloat16, order="C")
        )
    return [
        {"xt": xts[c // 2], "wa": was[c % 2], "wp": wps[c % 2], "msk": msk}
        for c in range(N_CORES)
    ]


def gather_output(results):
    out = np.empty((B, T, C), np.float32)
    for b in range(B):
        out[b] = results[2 * b]["outp"] + results[2 * b + 1]["outp"]
    return out


_RUNNER = None


def _make_cached_runner(nc):
    """jit the SPMD executable once so repeat kernel() calls skip the
    per-call retrace/recompile that run_bass_kernel_spmd pays."""
    import jax
    from jax.sharding import Mesh, PartitionSpec
    from jax.experimental.shard_map import shard_map
    from concourse import bass2jax

    bass2jax.install_neuronx_cc_hook()
    partition_name = nc.partition_id_tensor.name if nc.partition_id_tensor else None
    in_names, out_names, out_avals, zero_outs = [], [], [], []
    for alloc in nc.m.functions[0].allocations:
        if not isinstance(alloc, mybir.MemoryLocationSet):
            continue
        name = alloc.memorylocations[0].name
        if alloc.kind == "ExternalInput":
            if name != partition_name:
                in_names.append(name)
        elif alloc.kind == "ExternalOutput":
            shape = tuple(alloc.tensor_shape)
            dtype = mybir.dt.np(alloc.dtype)
            out_names.append(name)
            out_avals.append(jax.core.ShapedArray(shape, dtype))
            zero_outs.append(np.zeros(shape, dtype))
    n_params = len(in_names)
    n_outs = len(out_avals)
    all_in_names = in_names + out_names
    if partition_name is not None:
        all_in_names.append(partition_name)

    def _body(*args):
        operands = list(args)
        if partition_name is not None:
            operands.append(bass2jax.partition_id_tensor())
        return tuple(
            bass2jax._bass_exec_p.bind(
                *operands,
                out_avals=tuple(out_avals),
                in_names=tuple(all_in_names),
                out_names=tuple(out_names),
                lowering_input_output_aliases=(),
                sim_require_finite=True,
                sim_require_nnan=True,
                nc=nc,
            )
        )

    devices = jax.devices()[:N_CORES]
    mesh = Mesh(np.asarray(devices), ("core",))
    spec = PartitionSpec("core")
    sharded = jax.jit(
        shard_map(
            _body,
            mesh=mesh,
            in_specs=(spec,) * (n_params + n_outs),
            out_specs=(spec,) * n_outs,
            check_rep=False,
        ),
        donate_argnums=tuple(range(n_params, n_params + n_outs)),
        keep_unused=True,
    )

    def run(in_maps):
        concat_in = [
            np.concatenate([np.asarray(in_maps[c][nm]) for c in range(N_CORES)], 0)
            for nm in in_names
        ]
        zeros = [
            np.zeros((N_CORES * z.shape[0], *z.shape[1:]), z.dtype) for z in zero_outs
        ]
        outs = sharded(*concat_in, *zeros)
        return [
            {
                name: np.asarray(outs[i]).reshape(N_CORES, *out_avals[i].shape)[c]
                for i, name in enumerate(out_names)
            }
            for c in range(N_CORES)
        ]

    return run


def kernel(x, w_attn, w_proj):
    global _RUNNER
    x = np.asarray(x, dtype=np.float32)
    w_attn = np.asarray(w_attn, dtype=np.float32)
    w_proj = np.asarray(w_proj, dtype=np.float32)
    nc = _get_program()
    in_maps = make_in_maps(x, w_attn, w_proj)
    if _RUNNER is None:
        try:
            _RUNNER = _make_cached_runner(nc)
        except Exception:
            _RUNNER = None
        if _RUNNER is None:
            res = run_bass_kernel_spmd(nc, in_maps, core_ids=list(range(N_CORES)))
            return gather_output(res.results)
    try:
        return gather_output(_RUNNER(in_maps))
    except Exception:
        res = run_bass_kernel_spmd(nc, in_maps, core_ids=list(range(N_CORES)))
        return gather_output(res.results)
